# revision 14
# baseline (speedup 1.0000x reference)
"""Trainium2 kernel for nn_CircA (sign-flip -> r circular convolutions -> omega-mix).

The whole reference pipeline is linear in x:
    y[b,s,j] = sum_k x[b,s,k] * W[k,j] + bias[j]
    W[k,j]   = psi[k] * sum_r omega[r,j] * g[r, (j-k) mod d]
so the device work is a dense [8192,4096] @ [4096,4096] matmul. W is built
once on the host from the tiny psi/omega/g tensors; x is sharded row-wise
across the 8 NeuronCores (data parallel, no collectives) and transposed on
host so the contraction dim lands on SBUF partitions.

Per-core tiling: x^T shard [4096,1024] stays SBUF-resident; W streams through
SBUF in [128,512] tiles; one N-block (512 cols) of the output accumulates in
all 8 PSUM banks (one per 128-row M-tile) across the 32 K-tiles, then evicts
via DVE copy + DMA.
"""

import sys

import numpy as np

sys.path.insert(0, "/opt/trn_rl_repo")

D = 4096
R = 3
B, S = 4, 2048
M_TOTAL = B * S  # 8192
N_CORES = 8
M_CORE = M_TOTAL // N_CORES  # 1024
P = 128
NBLK = 512  # one PSUM bank of fp32
K_TILES = D // P  # 32
M_TILES = M_CORE // P  # 8
N_TILES = D // NBLK  # 8

# matmul operand dtype. Measured on HW (8-core SPMD, max-core exec time):
#   float32  ~4 cyc/row               (rel err ~1e-6,  ~1.7 ms)
#   float32r 227 ns/MM LDW-bound      (rel err 1.5e-4, 502 us)
#   float16  216 ns/MM stream-bound   (rel err 2.9e-4, 466 us)
#   bfloat16 222 ns/MM                (rel err 2.4e-3, 493 us)
DT = "float16"

_cache = {}


def _build_nc(dt_name):
    import concourse.mybir as mybir
    import concourse.tile as tile
    from concourse import bacc

    mdt = getattr(mybir.dt, dt_name)
    nc = bacc.Bacc(None, target_bir_lowering=False, debug=False)
    xt = nc.dram_tensor("xt", [D, M_CORE], mdt, kind="ExternalInput")
    w = nc.dram_tensor("w", [D, D], mdt, kind="ExternalInput")
    y = nc.dram_tensor("y", [M_CORE, D], mybir.dt.float32, kind="ExternalOutput")

    with tile.TileContext(nc) as tc:
        with tc.tile_pool(name="xt_sb", bufs=1) as xt_pool, \
             tc.tile_pool(name="w_sb", bufs=40) as w_pool, \
             tc.tile_pool(name="warm_sb", bufs=1) as warm_pool, \
             tc.tile_pool(name="ps", bufs=1, space="PSUM") as ps_pool, \
             tc.tile_pool(name="out_sb", bufs=6) as out_pool:
            xt_tiles = [
                xt_pool.tile([P, M_CORE], mdt, tag=f"xt{k}", name=f"xt{k}")
                for k in range(K_TILES)
            ]
            # HAM warmup: dummy matmuls keep the PE busy while the first
            # xt/w tiles stream in, so real matmuls start at 2.4 GHz.
            warm_f32 = warm_pool.tile([P, NBLK], mybir.dt.float32, tag="warm",
                                      name="warm")
            nc.vector.memset(warm_f32[:], 0.0)
            warm = warm_f32[:] if mdt == mybir.dt.float32 else \
                warm_f32[:].bitcast(mdt)
            for n in range(N_TILES):
                psums = [
                    ps_pool.tile([P, NBLK], mybir.dt.float32, tag=f"ps{m}",
                                 name=f"ps{m}_{n}")
                    for m in range(M_TILES)
                ]
                if n == 0:
                    # narrow (N=128) so the warmup drains before the first
                    # real tiles land, while still supplying ~3.5us of PE
                    # activity for the HAM un-throttle
                    for i in range(32):
                        nc.tensor.matmul(psums[i % M_TILES][:, :P],
                                         lhsT=warm[:, :P], rhs=warm[:, :P],
                                         start=True, stop=True,
                                         skip_group_check=True)
                if n < N_TILES - 1:
                    # k-inner: each streamed W tile feeds all 8 M-tiles, so
                    # the DMA stream (1 tile / 8 MMs) stays ahead of the PE
                    for k in range(K_TILES):
                        if n == 0:
                            # xt preload rides a separate DMA engine,
                            # interleaved with the first W sweep so k-step
                            # inputs land in order
                            nc.gpsimd.dma_start(
                                out=xt_tiles[k][:],
                                in_=xt.ap()[k * P:(k + 1) * P, :])
                        wt = w_pool.tile([P, NBLK], mdt, tag="w",
                                         name=f"w{n}_{k}")
                        nc.sync.dma_start(
                            out=wt[:],
                            in_=w.ap()[k * P:(k + 1) * P,
                                       n * NBLK:(n + 1) * NBLK])
                        for m in range(M_TILES):
                            nc.tensor.matmul(
                                psums[m][:],
                                lhsT=xt_tiles[k][:, m * P:(m + 1) * P],
                                rhs=wt[:],
                                start=(k == 0),
                                stop=(k == K_TILES - 1))
                    for m in range(M_TILES):
                        o = out_pool.tile([P, NBLK], mybir.dt.float32, tag="o",
                                          name=f"o{n}_{m}")
                        nc.vector.tensor_copy(o[:], psums[m][:])
                        nc.sync.dma_start(
                            out=y.ap()[m * P:(m + 1) * P,
                                       n * NBLK:(n + 1) * NBLK],
                            in_=o[:])
                else:
                    # Last block runs m-outer: the whole block's W is
                    # prefetched during block n-2/n-1 (bufs=40), each M-tile
                    # finishes its full k-sweep early, and 7 of the 8 PSUM
                    # evictions overlap remaining matmuls instead of
                    # serializing on DVE after the last one (~4us tail save).
                    wts = []
                    for k in range(K_TILES):
                        wt = w_pool.tile([P, NBLK], mdt, tag="w",
                                         name=f"w{n}_{k}")
                        nc.sync.dma_start(
                            out=wt[:],
                            in_=w.ap()[k * P:(k + 1) * P,
                                       n * NBLK:(n + 1) * NBLK])
                        wts.append(wt)
                    for m in range(M_TILES):
                        for k in range(K_TILES):
                            nc.tensor.matmul(
                                psums[m][:],
                                lhsT=xt_tiles[k][:, m * P:(m + 1) * P],
                                rhs=wts[k][:],
                                start=(k == 0),
                                stop=(k == K_TILES - 1))
                        o = out_pool.tile([P, NBLK], mybir.dt.float32, tag="o",
                                          name=f"o{n}_{m}")
                        nc.vector.tensor_copy(o[:], psums[m][:])
                        nc.sync.dma_start(
                            out=y.ap()[m * P:(m + 1) * P,
                                       n * NBLK:(n + 1) * NBLK],
                            in_=o[:])
    nc.compile()
    return nc


def _get_nc(dt_name):
    if dt_name not in _cache:
        _cache[dt_name] = _build_nc(dt_name)
    return _cache[dt_name]


def build_w(psi, omega, g):
    """W[k,j] = psi[k] * sum_r omega[r,j] * g[r,(j-k) mod d].

    Row k of the r-th circulant is g[r] rolled right by k, i.e.
    C_r[k, j] = gg[d - k + j] for gg = [g[r], g[r]] — expressed as a
    negative-stride view so no index matrix is materialized. float32
    accumulation errs ~1e-7, far below the fp16 operand rounding.
    """
    from numpy.lib.stride_tricks import as_strided
    w = np.zeros((D, D), dtype=np.float32)
    for r in range(R):
        gg = np.concatenate([g[r], g[r]]).astype(np.float32)
        s = gg.strides[0]
        circ = as_strided(gg[D:], shape=(D, D), strides=(-s, s))
        w += circ * omega[r].astype(np.float32)[None, :]
    w *= psi.reshape(-1, 1).astype(np.float32)
    return w


def _np_dtype(dt_name):
    if dt_name == "bfloat16":
        import ml_dtypes
        return np.dtype(ml_dtypes.bfloat16)
    if dt_name == "float16":
        return np.dtype(np.float16)
    return np.float32


def run_spmd(x, psi, omega, g, dt_name, trace=False):
    from concourse.bass_utils import run_bass_kernel_spmd

    nd = _np_dtype(dt_name)
    w_np = np.ascontiguousarray(build_w(psi, omega, g).astype(nd))
    xf = x.reshape(M_TOTAL, D)
    in_maps = []
    for c in range(N_CORES):
        shard = xf[c * M_CORE:(c + 1) * M_CORE]
        in_maps.append({
            "xt": np.ascontiguousarray(shard.T).astype(nd),
            "w": w_np,
        })
    nc = _get_nc(dt_name)
    res = run_bass_kernel_spmd(nc, in_maps, core_ids=list(range(N_CORES)),
                               trace=trace)
    y = np.empty((M_TOTAL, D), dtype=np.float32)
    for c in range(N_CORES):
        y[c * M_CORE:(c + 1) * M_CORE] = res.results[c]["y"]
    return y, res


def kernel(x, psi, omega, g, bias):
    x = np.asarray(x, dtype=np.float32)
    psi = np.asarray(psi, dtype=np.float32)
    omega = np.asarray(omega, dtype=np.float32)
    g = np.asarray(g, dtype=np.float32)
    bias = np.asarray(bias, dtype=np.float32)
    y, _ = run_spmd(x, psi, omega, g, DT)
    y = y + bias[None, :]
    return y.reshape(B, S, D)


# revision 16
# speedup vs baseline: 1.0017x; 1.0017x over previous
"""Trainium2 kernel for nn_CircA (sign-flip -> r circular convolutions -> omega-mix).

The whole reference pipeline is linear in x:
    y[b,s,j] = sum_k x[b,s,k] * W[k,j] + bias[j]
    W[k,j]   = psi[k] * sum_r omega[r,j] * g[r, (j-k) mod d]
so the device work is a dense [8192,4096] @ [4096,4096] matmul. W is built
once on the host from the tiny psi/omega/g tensors; x is sharded row-wise
across the 8 NeuronCores (data parallel, no collectives) and transposed on
host so the contraction dim lands on SBUF partitions.

Per-core tiling: x^T shard [4096,1024] stays SBUF-resident; W streams through
SBUF in [128,512] tiles; one N-block (512 cols) of the output accumulates in
all 8 PSUM banks (one per 128-row M-tile) across the 32 K-tiles, then evicts
via DVE copy + DMA.
"""

import sys

import numpy as np

sys.path.insert(0, "/opt/trn_rl_repo")

D = 4096
R = 3
B, S = 4, 2048
M_TOTAL = B * S  # 8192
N_CORES = 8
M_CORE = M_TOTAL // N_CORES  # 1024
P = 128
NBLK = 512  # one PSUM bank of fp32
K_TILES = D // P  # 32
M_TILES = M_CORE // P  # 8
N_TILES = D // NBLK  # 8

# matmul operand dtype. Measured on HW (8-core SPMD, max-core exec time):
#   float32  ~4 cyc/row               (rel err ~1e-6,  ~1.7 ms)
#   float32r 227 ns/MM LDW-bound      (rel err 1.5e-4, 502 us)
#   float16  216 ns/MM stream-bound   (rel err 2.9e-4, 466 us)
#   bfloat16 222 ns/MM                (rel err 2.4e-3, 493 us)
DT = "float16"

_cache = {}


def _build_nc(dt_name):
    import concourse.mybir as mybir
    import concourse.tile as tile
    from concourse import bacc

    mdt = getattr(mybir.dt, dt_name)
    nc = bacc.Bacc(None, target_bir_lowering=False, debug=False)
    xt = nc.dram_tensor("xt", [D, M_CORE], mdt, kind="ExternalInput")
    w = nc.dram_tensor("w", [D, D], mdt, kind="ExternalInput")
    y = nc.dram_tensor("y", [M_CORE, D], mybir.dt.float32, kind="ExternalOutput")

    with tile.TileContext(nc) as tc:
        with tc.tile_pool(name="xt_sb", bufs=1) as xt_pool, \
             tc.tile_pool(name="w_sb", bufs=40) as w_pool, \
             tc.tile_pool(name="warm_sb", bufs=1) as warm_pool, \
             tc.tile_pool(name="ps", bufs=1, space="PSUM") as ps_pool, \
             tc.tile_pool(name="out_sb", bufs=6) as out_pool:
            xt_tiles = [
                xt_pool.tile([P, M_CORE], mdt, tag=f"xt{k}", name=f"xt{k}")
                for k in range(K_TILES)
            ]
            # HAM warmup: dummy matmuls keep the PE busy while the first
            # xt/w tiles stream in, so real matmuls start at 2.4 GHz. Kept
            # small ([P,P]) so the prerequisite memset clears the DVE as
            # early as possible after the engine preamble.
            warm_t = warm_pool.tile([P, P], mdt, tag="warm", name="warm")
            nc.vector.memset(warm_t[:], 0.0)
            warm = warm_t[:]
            for n in range(N_TILES):
                psums = [
                    ps_pool.tile([P, NBLK], mybir.dt.float32, tag=f"ps{m}",
                                 name=f"ps{m}_{n}")
                    for m in range(M_TILES)
                ]
                if n == 0:
                    # narrow (N=128) so the warmup drains before the first
                    # real tiles land, while still supplying ~3.5us of PE
                    # activity for the HAM un-throttle
                    for i in range(32):
                        nc.tensor.matmul(psums[i % M_TILES][:, :P],
                                         lhsT=warm[:, :P], rhs=warm[:, :P],
                                         start=True, stop=True,
                                         skip_group_check=True)
                if n < N_TILES - 1:
                    # k-inner: each streamed W tile feeds all 8 M-tiles, so
                    # the DMA stream (1 tile / 8 MMs) stays ahead of the PE
                    for k in range(K_TILES):
                        if n == 0:
                            # xt preload rides a separate DMA engine,
                            # interleaved with the first W sweep so k-step
                            # inputs land in order
                            nc.gpsimd.dma_start(
                                out=xt_tiles[k][:],
                                in_=xt.ap()[k * P:(k + 1) * P, :])
                        wt = w_pool.tile([P, NBLK], mdt, tag="w",
                                         name=f"w{n}_{k}")
                        nc.sync.dma_start(
                            out=wt[:],
                            in_=w.ap()[k * P:(k + 1) * P,
                                       n * NBLK:(n + 1) * NBLK])
                        for m in range(M_TILES):
                            nc.tensor.matmul(
                                psums[m][:],
                                lhsT=xt_tiles[k][:, m * P:(m + 1) * P],
                                rhs=wt[:],
                                start=(k == 0),
                                stop=(k == K_TILES - 1))
                    for m in range(M_TILES):
                        o = out_pool.tile([P, NBLK], mybir.dt.float32, tag="o",
                                          name=f"o{n}_{m}")
                        nc.vector.tensor_copy(o[:], psums[m][:])
                        nc.sync.dma_start(
                            out=y.ap()[m * P:(m + 1) * P,
                                       n * NBLK:(n + 1) * NBLK],
                            in_=o[:])
                else:
                    # Last block runs m-outer: the whole block's W is
                    # prefetched during block n-2/n-1 (bufs=40), each M-tile
                    # finishes its full k-sweep early, and 7 of the 8 PSUM
                    # evictions overlap remaining matmuls instead of
                    # serializing on DVE after the last one (~4us tail save).
                    wts = []
                    for k in range(K_TILES):
                        wt = w_pool.tile([P, NBLK], mdt, tag="w",
                                         name=f"w{n}_{k}")
                        nc.sync.dma_start(
                            out=wt[:],
                            in_=w.ap()[k * P:(k + 1) * P,
                                       n * NBLK:(n + 1) * NBLK])
                        wts.append(wt)
                    for m in range(M_TILES):
                        for k in range(K_TILES):
                            nc.tensor.matmul(
                                psums[m][:],
                                lhsT=xt_tiles[k][:, m * P:(m + 1) * P],
                                rhs=wts[k][:],
                                start=(k == 0),
                                stop=(k == K_TILES - 1))
                        o = out_pool.tile([P, NBLK], mybir.dt.float32, tag="o",
                                          name=f"o{n}_{m}")
                        nc.vector.tensor_copy(o[:], psums[m][:])
                        nc.sync.dma_start(
                            out=y.ap()[m * P:(m + 1) * P,
                                       n * NBLK:(n + 1) * NBLK],
                            in_=o[:])
    nc.compile()
    return nc


def _get_nc(dt_name):
    if dt_name not in _cache:
        _cache[dt_name] = _build_nc(dt_name)
    return _cache[dt_name]


def build_w(psi, omega, g):
    """W[k,j] = psi[k] * sum_r omega[r,j] * g[r,(j-k) mod d].

    Row k of the r-th circulant is g[r] rolled right by k, i.e.
    C_r[k, j] = gg[d - k + j] for gg = [g[r], g[r]] — expressed as a
    negative-stride view so no index matrix is materialized. float32
    accumulation errs ~1e-7, far below the fp16 operand rounding.
    """
    from numpy.lib.stride_tricks import as_strided
    w = np.zeros((D, D), dtype=np.float32)
    for r in range(R):
        gg = np.concatenate([g[r], g[r]]).astype(np.float32)
        s = gg.strides[0]
        circ = as_strided(gg[D:], shape=(D, D), strides=(-s, s))
        w += circ * omega[r].astype(np.float32)[None, :]
    w *= psi.reshape(-1, 1).astype(np.float32)
    return w


def _np_dtype(dt_name):
    if dt_name == "bfloat16":
        import ml_dtypes
        return np.dtype(ml_dtypes.bfloat16)
    if dt_name == "float16":
        return np.dtype(np.float16)
    return np.float32


def run_spmd(x, psi, omega, g, dt_name, trace=False):
    from concourse.bass_utils import run_bass_kernel_spmd

    nd = _np_dtype(dt_name)
    w_np = np.ascontiguousarray(build_w(psi, omega, g).astype(nd))
    xf = x.reshape(M_TOTAL, D)
    in_maps = []
    for c in range(N_CORES):
        shard = xf[c * M_CORE:(c + 1) * M_CORE]
        in_maps.append({
            "xt": np.ascontiguousarray(shard.T).astype(nd),
            "w": w_np,
        })
    nc = _get_nc(dt_name)
    res = run_bass_kernel_spmd(nc, in_maps, core_ids=list(range(N_CORES)),
                               trace=trace)
    y = np.empty((M_TOTAL, D), dtype=np.float32)
    for c in range(N_CORES):
        y[c * M_CORE:(c + 1) * M_CORE] = res.results[c]["y"]
    return y, res


def kernel(x, psi, omega, g, bias):
    x = np.asarray(x, dtype=np.float32)
    psi = np.asarray(psi, dtype=np.float32)
    omega = np.asarray(omega, dtype=np.float32)
    g = np.asarray(g, dtype=np.float32)
    bias = np.asarray(bias, dtype=np.float32)
    y, _ = run_spmd(x, psi, omega, g, DT)
    y = y + bias[None, :]
    return y.reshape(B, S, D)
